# revision 5
# baseline (speedup 1.0000x reference)
"""AlignmentEncoder (retrieval_knn) Trainium2 kernel, 8-core data-parallel.

Math (per batch):
  k~ = conv1d(conv1d(emb[keys], kW1, k=3) relu, kW2, k=1)        [T2, 80]
  q~ = conv3(relu(conv2(relu(conv1(queries)))))                   [T1, 80]
  s[t1,t2] = -T*(||q~-k~||^2)  -> log_softmax over t2 cancels the
  q~^2 term, so s := 2T*(q~.k~) - T*||k~||^2 suffices.
  Fold conv3 into the key side: q~.k~ = h2 . (W3 @ k~^T), so the
  T1-sized path stops at h2 (conv2 output) and the s-matmul contracts
  h2aug=[h2;1] (81 rows) against kaug=[2T*W3k~ ; 2T*qb3.k~-T*k2].
  out1 = s - lse + ln(prior+1e-8) = ln( exp(s) * priorp / sum_e )
  out2 = softmax(out1 [masked]) = w / sum(w),  w = exp(s)*priorp*r1

Per core: 4 batches; softmax stage processes batch-pairs with t2 maps
of [128, 2*512] so DMAs and the Ln pass run at free-dim 1024.
"""
import numpy as np
import ml_dtypes

BF16 = ml_dtypes.bfloat16

B, T1, T2 = 32, 2048, 512
C_MEL, C_ATT, EMB, VOCAB = 80, 80, 512, 256
C1 = 1024          # key conv1 output channels (2*C_TXT)
CQ1 = 160          # query conv1 output channels (2*C_MEL)
TEMP = 0.0005
NCORES = 8
BL = B // NCORES   # batches per core
NPAIR = BL // 2
NM = T1 // 128     # t1 tiles per batch

_cache = {}


def _patch_act_tables():
    """Force every ACT function onto the one table set that has them all
    (exp/ln/relu/copy/square), so the compiler emits a single table load
    instead of thrashing 2.7us loads between Exp and Ln."""
    import concourse.hw_specs as hw_specs
    import concourse.bacc as bacc
    keep = "natural_log_exp_and_others"
    real = hw_specs.get_activation_tables

    def only_keep(arch):
        tabs = real(arch)
        return {k: (v if k == keep else set()) for k, v in tabs.items()}

    bacc.get_activation_tables = only_keep


def _build(any_masked: bool):
    import concourse.bacc as bacc
    import concourse.mybir as mybir
    from concourse.tile import TileContext

    _patch_act_tables()

    dt = mybir.dt
    AF = mybir.ActivationFunctionType
    OP = mybir.AluOpType

    nc = bacc.Bacc("TRN2", target_bir_lowering=False, debug=False,
                   num_devices=NCORES)

    def din(name, shape, dtype=dt.bfloat16):
        return nc.dram_tensor(name, shape, dtype, kind="ExternalInput")

    ecm = din("ecm", [BL, 4, 128, T2 + 2])
    qTd = din("qT", [BL, C_MEL, T1 + 2])
    ppd = din("priorp", [NPAIR, NM, 128, 2, T2])
    pmd = din("pm", [NPAIR, NM, 128, 2, T2]) if any_masked else None
    kW1d = din("kW1", [3, 4, 128, C1])
    kW2d = din("kW2", [8, 128, C_ATT])
    W3d = din("W3s", [C_ATT, C_ATT])
    qW1d = din("qW1", [3, C_MEL, CQ1])
    qW2d = din("qW2", [2, C_MEL, C_MEL])
    qb3d = din("qb3s", [C_ATT, 1])
    kb1d = din("kb1", [128, 8], dt.float32)
    kb2d = din("kb2", [C_ATT, 1], dt.float32)
    qb1d = din("qb1", [C_MEL, 2], dt.float32)
    qb2d = din("qb2", [C_MEL, 1], dt.float32)

    o1d = nc.dram_tensor("out1", [NPAIR, NM, 128, 2, T2], dt.bfloat16,
                         kind="ExternalOutput")
    o2d = nc.dram_tensor("out2", [NPAIR, NM, 128, 2, T2], dt.bfloat16,
                         kind="ExternalOutput")

    with TileContext(nc) as tc:
        import contextlib
        with contextlib.ExitStack() as ctx:
            wpool = ctx.enter_context(tc.tile_pool(name="weights", bufs=1))
            ekpool = ctx.enter_context(tc.tile_pool(name="ek", bufs=2))
            h1kpool = ctx.enter_context(tc.tile_pool(name="h1k", bufs=2))
            kaugpool = ctx.enter_context(tc.tile_pool(name="kaug", bufs=4))
            qpool = ctx.enter_context(tc.tile_pool(name="qp", bufs=2))
            h2pool = ctx.enter_context(tc.tile_pool(name="h2", bufs=4))
            iopool = ctx.enter_context(tc.tile_pool(name="io", bufs=3))
            stat = ctx.enter_context(tc.tile_pool(name="stat", bufs=4))
            cpool = ctx.enter_context(
                tc.tile_pool(name="cps", bufs=4, space="PSUM"))
            spsum = ctx.enter_context(
                tc.tile_pool(name="sps", bufs=2, space="PSUM"))

            # ---- persistent weights/biases in SBUF ----
            kW1sb = {}
            for d in range(3):
                for k in range(4):
                    t = wpool.tile([128, C1], dt.bfloat16, tag=f"kW1_{d}_{k}")
                    nc.sync.dma_start(out=t[:], in_=kW1d[d, k])
                    kW1sb[d, k] = t
            kW2sb = wpool.tile([128, 8 * C_ATT], dt.bfloat16, tag="kW2")
            for k in range(8):
                nc.sync.dma_start(out=kW2sb[:, k * C_ATT:(k + 1) * C_ATT],
                                  in_=kW2d[k])
            W3sb = wpool.tile([C_ATT, C_ATT], dt.bfloat16, tag="W3")
            nc.sync.dma_start(out=W3sb[:], in_=W3d[:])
            qW1sb = wpool.tile([C_MEL, 3 * CQ1], dt.bfloat16, tag="qW1")
            for d in range(3):
                nc.sync.dma_start(out=qW1sb[:, d * CQ1:(d + 1) * CQ1],
                                  in_=qW1d[d])
            qW2sb = wpool.tile([C_MEL, 2 * C_MEL], dt.bfloat16, tag="qW2")
            for k in range(2):
                nc.sync.dma_start(out=qW2sb[:, k * C_MEL:(k + 1) * C_MEL],
                                  in_=qW2d[k])
            qb3sb = wpool.tile([C_ATT, 1], dt.bfloat16, tag="qb3")
            nc.sync.dma_start(out=qb3sb[:], in_=qb3d[:])
            negT = wpool.tile([C_ATT, 1], dt.bfloat16, tag="negT")
            nc.vector.memset(negT[:], -TEMP)
            kb1sb = wpool.tile([128, 8], dt.float32, tag="kb1")
            nc.sync.dma_start(out=kb1sb[:], in_=kb1d[:])
            kb2sb = wpool.tile([C_ATT, 1], dt.float32, tag="kb2")
            nc.sync.dma_start(out=kb2sb[:], in_=kb2d[:])
            qb1sb = wpool.tile([C_MEL, 2], dt.float32, tag="qb1")
            nc.sync.dma_start(out=qb1sb[:], in_=qb1d[:])
            qb2sb = wpool.tile([C_MEL, 1], dt.float32, tag="qb2")
            nc.sync.dma_start(out=qb2sb[:], in_=qb2d[:])

            kaug = {}
            h2aug = {}
            relu_flip = [0]

            def relu_copy(dst, src, bias_ap):
                """PSUM->SBUF relu+bias, alternating ACT/DVE for balance."""
                relu_flip[0] ^= 1
                if relu_flip[0]:
                    nc.scalar.activation(dst, src, AF.Relu, bias=bias_ap)
                else:
                    nc.vector.tensor_scalar(dst, src, bias_ap, 0.0,
                                            OP.add, OP.max)

            def key_path(b):
                ek = []
                for k in range(4):
                    t = ekpool.tile([128, T2 + 2], dt.bfloat16, tag=f"ek{k}")
                    nc.sync.dma_start(out=t[:], in_=ecm[b, k])
                    ek.append(t)
                h1k = []
                for m in range(8):
                    ps = cpool.tile([128, T2], mybir.dt.float32, tag="cps")
                    i = 0
                    for d in range(3):
                        for k in range(4):
                            nc.tensor.matmul(
                                ps[:],
                                kW1sb[d, k][:, m * 128:(m + 1) * 128],
                                ek[k][:, d:d + T2],
                                start=(i == 0), stop=(i == 11))
                            i += 1
                    h = h1kpool.tile([128, T2], dt.bfloat16, tag=f"h1k{m}")
                    relu_copy(h[:], ps[:], kb1sb[:, m:m + 1])
                    h1k.append(h)
                ps2 = cpool.tile([C_ATT, T2], mybir.dt.float32, tag="cps")
                for k in range(8):
                    nc.tensor.matmul(ps2[:],
                                     kW2sb[:, k * C_ATT:(k + 1) * C_ATT],
                                     h1k[k][:], start=(k == 0), stop=(k == 7))
                ksb = stat.tile([C_ATT, T2], dt.bfloat16, tag="ksb")
                nc.scalar.activation(ksb[:], ps2[:], AF.Identity,
                                     bias=kb2sb[:])
                ps3 = cpool.tile([C_ATT, T2], mybir.dt.float32, tag="cps")
                nc.tensor.matmul(ps3[:], W3sb[:], ksb[:], start=True,
                                 stop=True)
                sq = stat.tile([C_ATT, T2], dt.bfloat16, tag="sq")
                nc.vector.tensor_tensor(sq[:], ksb[:], ksb[:], OP.mult)
                psr = cpool.tile([1, T2], mybir.dt.float32, tag="cps")
                nc.tensor.matmul(psr[:], negT[:], sq[:], start=True,
                                 stop=False)
                nc.tensor.matmul(psr[:], qb3sb[:], ksb[:], start=False,
                                 stop=True)
                # augmented row lives at partition 96 (32-aligned base);
                # rows 80..95 zeroed so they contribute nothing to the
                # K=97 contraction.
                ka = kaugpool.tile([97, T2], dt.bfloat16, tag="kaug")
                nc.vector.memset(ka[64:96, :], 0.0)
                nc.scalar.activation(ka[0:C_ATT, :], ps3[:], AF.Copy)
                nc.scalar.activation(ka[96:97, :], psr[:], AF.Copy)
                kaug[b] = ka

            def query_path(b):
                qsb = qpool.tile([C_MEL, T1 + 2], dt.bfloat16, tag="qsb")
                nc.sync.dma_start(out=qsb[:], in_=qTd[b])
                h2 = h2pool.tile([97, T1], dt.bfloat16, tag="h2aug")
                nc.vector.memset(h2[64:96, :], 0.0)
                nc.vector.memset(h2[96:97, :], 1.0)
                for c in range(4):
                    h1q = []
                    for mi in range(2):
                        ps = cpool.tile([C_MEL, T2], mybir.dt.float32,
                                        tag="cps")
                        for d in range(3):
                            nc.tensor.matmul(
                                ps[:],
                                qW1sb[:, d * CQ1 + mi * C_MEL:
                                      d * CQ1 + (mi + 1) * C_MEL],
                                qsb[:, c * T2 + d:c * T2 + d + T2],
                                start=(d == 0), stop=(d == 2))
                        h = qpool.tile([C_MEL, T2], dt.bfloat16,
                                       tag=f"h1q{mi}")
                        relu_copy(h[:], ps[:], qb1sb[:, mi:mi + 1])
                        h1q.append(h)
                    ps2 = cpool.tile([C_MEL, T2], mybir.dt.float32, tag="cps")
                    for mi in range(2):
                        nc.tensor.matmul(
                            ps2[:],
                            qW2sb[:, mi * C_MEL:(mi + 1) * C_MEL],
                            h1q[mi][:], start=(mi == 0), stop=(mi == 1))
                    relu_copy(h2[0:C_ATT, c * T2:(c + 1) * T2], ps2[:],
                              qb2sb[:])
                h2aug[b] = h2

            def softmax_tile(p, m):
                b0, b1 = 2 * p, 2 * p + 1
                sps = spsum.tile([128, 2, T2], mybir.dt.float32, tag="sps")
                nc.tensor.matmul(sps[:, 0], h2aug[b0][:, m * 128:(m + 1) * 128],
                                 kaug[b0][:], start=True, stop=True)
                nc.tensor.matmul(sps[:, 1], h2aug[b1][:, m * 128:(m + 1) * 128],
                                 kaug[b1][:], start=True, stop=True)
                pp = iopool.tile([128, 2, T2], dt.bfloat16, tag="pp")
                nc.sync.dma_start(out=pp[:], in_=ppd[p, m])
                et = iopool.tile([128, 2, T2], dt.bfloat16, tag="et")
                sums = stat.tile([128, 2], mybir.dt.float32, tag="sume")
                for j in range(2):
                    nc.scalar.activation(et[:, j], sps[:, j], AF.Exp,
                                         accum_out=sums[:, j:j + 1])
                r1 = stat.tile([128, 2], mybir.dt.float32, tag="r1")
                nc.vector.reciprocal(r1[:], sums[:])
                wt = iopool.tile([128, 2, T2], dt.bfloat16, tag="wt")
                sums2 = stat.tile([128, 2], mybir.dt.float32, tag="sumw")
                for j in range(2):
                    nc.vector.scalar_tensor_tensor(
                        wt[:, j], et[:, j], r1[:, j:j + 1], pp[:, j],
                        OP.mult, OP.mult,
                        accum_out=None if any_masked else sums2[:, j:j + 1])
                o1 = iopool.tile([128, 2, T2], dt.bfloat16, tag="o1")
                nc.scalar.activation(o1[:], wt[:], AF.Ln)
                nc.sync.dma_start(out=o1d[p, m], in_=o1[:])
                if any_masked:
                    pm = iopool.tile([128, 2, T2], dt.bfloat16, tag="pmt")
                    nc.sync.dma_start(out=pm[:], in_=pmd[p, m])
                    wm = iopool.tile([128, 2, T2], dt.bfloat16, tag="wm")
                    for j in range(2):
                        nc.vector.scalar_tensor_tensor(
                            wm[:, j], et[:, j], r1[:, j:j + 1], pm[:, j],
                            OP.mult, OP.mult, accum_out=sums2[:, j:j + 1])
                    wsrc = wm
                else:
                    wsrc = wt
                r2 = stat.tile([128, 2], mybir.dt.float32, tag="r2")
                nc.vector.reciprocal(r2[:], sums2[:])
                o2 = iopool.tile([128, 2, T2], dt.bfloat16, tag="o2")
                for j in range(2):
                    nc.vector.tensor_scalar(o2[:, j], wsrc[:, j],
                                            r2[:, j:j + 1], None, OP.mult)
                nc.sync.dma_start(out=o2d[p, m], in_=o2[:])

            # ---- schedule ----
            key_path(0)
            query_path(0)
            key_path(1)
            query_path(1)
            # interleave pair-0 softmax with batches 2/3 compute so ACT/DVE
            # softmax work overlaps PE conv work
            key_path(2)
            for m in range(8):
                softmax_tile(0, m)
            query_path(2)
            key_path(3)
            for m in range(8, 16):
                softmax_tile(0, m)
            query_path(3)
            for m in range(16):
                softmax_tile(1, m)

    nc.compile()
    return nc


def _prep(inputs):
    """Host-side shard prep. Returns (in_maps, any_masked)."""
    queries = np.asarray(inputs["queries"], np.float32)
    keys = np.asarray(inputs["keys"])
    mask = np.asarray(inputs["mask"]).astype(bool)
    prior = np.asarray(inputs["attn_prior"], np.float32)
    emb = np.asarray(inputs["emb"], np.float32)
    kW1 = np.asarray(inputs["kW1"], np.float32)
    kb1 = np.asarray(inputs["kb1"], np.float32)
    kW2 = np.asarray(inputs["kW2"], np.float32)
    kb2 = np.asarray(inputs["kb2"], np.float32)
    qW1 = np.asarray(inputs["qW1"], np.float32)
    qb1 = np.asarray(inputs["qb1"], np.float32)
    qW2 = np.asarray(inputs["qW2"], np.float32)
    qb2 = np.asarray(inputs["qb2"], np.float32)
    qW3 = np.asarray(inputs["qW3"], np.float32)
    qb3 = np.asarray(inputs["qb3"], np.float32)

    any_masked = not mask.all()

    # weights (shared by all cores)
    kW1s = np.ascontiguousarray(
        kW1.reshape(3, 4, 128, C1)).astype(BF16)          # [d][k] lhsT
    kW2s = np.ascontiguousarray(
        kW2[0].reshape(8, 128, C_ATT)).astype(BF16)
    W3s = np.ascontiguousarray((2.0 * TEMP) * qW3[0].T).astype(BF16)
    qW1s = np.ascontiguousarray(qW1).astype(BF16)          # [3, 80, 160]
    qW2s = np.ascontiguousarray(
        qW2[0].reshape(2, C_MEL, C_MEL)).astype(BF16)
    qb3s = ((2.0 * TEMP) * qb3).reshape(C_ATT, 1).astype(BF16)
    kb1s = np.ascontiguousarray(
        kb1.reshape(8, 128).T).astype(np.float32)          # [128, 8]
    kb2s = kb2.reshape(C_ATT, 1).astype(np.float32)
    qb1s = np.ascontiguousarray(
        qb1.reshape(2, C_MEL).T).astype(np.float32)        # [80, 2]
    qb2s = qb2.reshape(C_MEL, 1).astype(np.float32)

    priorp = prior + 1e-8
    shared = dict(kW1=kW1s, kW2=kW2s, W3s=W3s, qW1=qW1s, qW2=qW2s,
                  qb3s=qb3s, kb1=kb1s, kb2=kb2s, qb1=qb1s, qb2=qb2s)

    in_maps = []
    for i in range(NCORES):
        bs = slice(BL * i, BL * (i + 1))
        # embedding gather, channel-major, zero halo on t2
        e = emb[keys[bs]]                        # [BL, T2, EMB]
        e_cm = np.zeros((BL, EMB, T2 + 2), np.float32)
        e_cm[:, :, 1:T2 + 1] = e.transpose(0, 2, 1)
        ecm = np.ascontiguousarray(
            e_cm.reshape(BL, 4, 128, T2 + 2)).astype(BF16)
        # wait: reshape must split EMB (axis 1) into 4x128 -> OK since
        # e_cm is [BL, EMB, T2+2] contiguous.
        qT = np.zeros((BL, C_MEL, T1 + 2), np.float32)
        qT[:, :, 1:T1 + 1] = queries[bs].transpose(0, 2, 1)
        qTs = qT.astype(BF16)
        # priorp paired layout [NPAIR, NM, 128, 2, T2]
        pp = priorp[bs].reshape(NPAIR, 2, NM, 128, T2)
        pp = np.ascontiguousarray(pp.transpose(0, 2, 3, 1, 4)).astype(BF16)
        m = dict(ecm=ecm, qT=qTs, priorp=pp, **shared)
        if any_masked:
            pmv = priorp[bs] * mask[bs, :, 0][:, None, :]
            pmv = pmv.reshape(NPAIR, 2, NM, 128, T2)
            m["pm"] = np.ascontiguousarray(
                pmv.transpose(0, 2, 3, 1, 4)).astype(BF16)
        in_maps.append(m)
    return in_maps, any_masked


def _assemble(results):
    out1 = np.empty((B, 1, T1, T2), np.float32)
    out2 = np.empty((B, 1, T1, T2), np.float32)
    for i, r in enumerate(results):
        for name, dst in (("out1", out1), ("out2", out2)):
            a = np.asarray(r[name]).astype(np.float32)
            a = a.reshape(NPAIR, NM, 128, 2, T2).transpose(0, 3, 1, 2, 4)
            dst[BL * i:BL * (i + 1), 0] = a.reshape(BL, T1, T2)
    return out2, out1


def kernel(**inputs):
    from concourse import bass_utils

    in_maps, any_masked = _prep(inputs)
    key = any_masked
    if key not in _cache:
        _cache[key] = _build(any_masked)
    nc = _cache[key]
    res = bass_utils.run_bass_kernel_spmd(
        nc, in_maps, core_ids=list(range(NCORES)))
    return _assemble(res.results)


# revision 7
# speedup vs baseline: 1.2387x; 1.2387x over previous
"""AlignmentEncoder (retrieval_knn) Trainium2 kernel, 8-core data-parallel.

Math (per batch):
  k~ = conv1d_k1(relu(conv1d_k3(emb[keys])))                      [T2, 80]
  distance logits after log_softmax-constant cancellation:
    s[t1,t2] = 2T*(q~.k~) - T*||k~||^2   (q~^2 term cancels)
  conv3 of the query path is folded into the key side:
    q~.k~ = h2 . (W3 @ k~^T), so the T1-sized path stops at h2 and the
    s-matmul contracts h2aug=[h2;0;1] (97 rows, ones row at partition
    96 for alignment) against kaug=[2T*W3k~ ; 0 ; 2T*qb3.k~ - T*k2].
  out1 = s - lse + ln(prior+1e-8) = ln( exp(s) * priorp / sum_e )
  out2 = softmax over t2 = w / sum(w),  w = exp(s)*priorp*r1

Per core: 4 batches; softmax processes adjacent t1-tile pairs (m, m+1)
of one batch as [128, 2*512] so the Ln pass and DMAs run at free-dim
1024. Softmax of batch b is interleaved with batch b+1's convs.
"""
import numpy as np
import ml_dtypes

BF16 = ml_dtypes.bfloat16

B, T1, T2 = 32, 2048, 512
C_MEL, C_ATT, EMB, VOCAB = 80, 80, 512, 256
C1 = 1024          # key conv1 output channels (2*C_TXT)
CQ1 = 160          # query conv1 output channels (2*C_MEL)
TEMP = 0.0005
NCORES = 8
BL = B // NCORES   # batches per core
NM = T1 // 128     # t1 tiles per batch

_cache = {}


def _patch_act_tables():
    """Force every ACT function onto the one table set that has them all
    (exp/ln/relu/copy/square), so the compiler emits a single table load
    instead of thrashing 2.7us loads between Exp and Ln."""
    import concourse.hw_specs as hw_specs
    import concourse.bacc as bacc
    keep = "natural_log_exp_and_others"
    real = hw_specs.get_activation_tables

    def only_keep(arch):
        tabs = real(arch)
        return {k: (v if k == keep else set()) for k, v in tabs.items()}

    bacc.get_activation_tables = only_keep


def _build(any_masked: bool):
    import contextlib

    import concourse.bacc as bacc
    import concourse.mybir as mybir
    from concourse.tile import TileContext

    _patch_act_tables()

    dt = mybir.dt
    AF = mybir.ActivationFunctionType
    OP = mybir.AluOpType
    f32 = mybir.dt.float32

    nc = bacc.Bacc("TRN2", target_bir_lowering=False, debug=False,
                   num_devices=NCORES)

    def din(name, shape, dtype=dt.bfloat16):
        return nc.dram_tensor(name, shape, dtype, kind="ExternalInput")

    ecm = din("ecm", [BL, 128, 4 * (T2 + 2)])
    qTd = din("qT", [BL, C_MEL, T1 + 2])
    ppd = din("priorp", [BL, NM // 2, 128, 2, T2])
    pmd = din("pm", [BL, NM // 2, 128, 2, T2]) if any_masked else None
    kW1d = din("kW1", [128, 12 * C1])
    kW2d = din("kW2", [128, 8 * C_ATT])
    W3d = din("W3s", [C_ATT, C_ATT])
    qW1d = din("qW1", [C_MEL, 3 * CQ1])
    qW2d = din("qW2", [C_MEL, 2 * C_MEL])
    qb3d = din("qb3s", [C_ATT, 1])
    kb1d = din("kb1", [128, 8], f32)
    kb2d = din("kb2", [C_ATT, 1], f32)
    qb1d = din("qb1", [C_MEL, 2], f32)
    qb2d = din("qb2", [C_MEL, 1], f32)

    o1d = nc.dram_tensor("out1", [BL, NM // 2, 128, 2, T2], dt.bfloat16,
                         kind="ExternalOutput")
    o2d = nc.dram_tensor("out2", [BL, NM // 2, 128, 2, T2], dt.bfloat16,
                         kind="ExternalOutput")

    with TileContext(nc) as tc:
        with contextlib.ExitStack() as ctx:
            wpool = ctx.enter_context(tc.tile_pool(name="weights", bufs=1))
            ekpool = ctx.enter_context(tc.tile_pool(name="ek", bufs=2))
            h1kpool = ctx.enter_context(tc.tile_pool(name="h1k", bufs=2))
            kaugpool = ctx.enter_context(tc.tile_pool(name="kaug", bufs=2))
            qpool = ctx.enter_context(tc.tile_pool(name="qp", bufs=2))
            h2pool = ctx.enter_context(tc.tile_pool(name="h2", bufs=2))
            iopool = ctx.enter_context(tc.tile_pool(name="io", bufs=3))
            stat = ctx.enter_context(tc.tile_pool(name="stat", bufs=4))
            cpool = ctx.enter_context(
                tc.tile_pool(name="cps", bufs=4, space="PSUM"))
            spsum = ctx.enter_context(
                tc.tile_pool(name="sps", bufs=2, space="PSUM"))

            # ---- persistent weights/biases, one batched DMA each ----
            kW1sb = wpool.tile([128, 12 * C1], dt.bfloat16, tag="kW1")
            nc.sync.dma_start(out=kW1sb[:], in_=kW1d[:])
            kW2sb = wpool.tile([128, 8 * C_ATT], dt.bfloat16, tag="kW2")
            nc.sync.dma_start(out=kW2sb[:], in_=kW2d[:])
            W3sb = wpool.tile([C_ATT, C_ATT], dt.bfloat16, tag="W3")
            nc.sync.dma_start(out=W3sb[:], in_=W3d[:])
            qW1sb = wpool.tile([C_MEL, 3 * CQ1], dt.bfloat16, tag="qW1")
            nc.sync.dma_start(out=qW1sb[:], in_=qW1d[:])
            qW2sb = wpool.tile([C_MEL, 2 * C_MEL], dt.bfloat16, tag="qW2")
            nc.sync.dma_start(out=qW2sb[:], in_=qW2d[:])
            qb3sb = wpool.tile([C_ATT, 1], dt.bfloat16, tag="qb3")
            nc.sync.dma_start(out=qb3sb[:], in_=qb3d[:])
            negT = wpool.tile([C_ATT, 1], dt.bfloat16, tag="negT")
            nc.gpsimd.memset(negT[:], -TEMP)
            kb1sb = wpool.tile([128, 8], f32, tag="kb1")
            nc.sync.dma_start(out=kb1sb[:], in_=kb1d[:])
            kb2sb = wpool.tile([C_ATT, 1], f32, tag="kb2")
            nc.sync.dma_start(out=kb2sb[:], in_=kb2d[:])
            qb1sb = wpool.tile([C_MEL, 2], f32, tag="qb1")
            nc.sync.dma_start(out=qb1sb[:], in_=qb1d[:])
            qb2sb = wpool.tile([C_MEL, 1], f32, tag="qb2")
            nc.sync.dma_start(out=qb2sb[:], in_=qb2d[:])

            def kw1(d, k, m):
                off = (d * 4 + k) * C1
                return kW1sb[:, off + m * 128:off + (m + 1) * 128]

            kaug = {}
            h2aug = {}
            relu_cnt = [0]

            def relu_copy(dst, src, bias_ap):
                """PSUM->SBUF relu+bias; ~1/3 on ACT, 2/3 on DVE."""
                relu_cnt[0] += 1
                if relu_cnt[0] % 3 == 0:
                    nc.scalar.activation(dst, src, AF.Relu, bias=bias_ap)
                else:
                    nc.vector.tensor_scalar(dst, src, bias_ap, 0.0,
                                            OP.add, OP.max)

            def conv_units(b):
                """Yield schedulable units of batch b's conv work."""
                def u_ek():
                    ek = ekpool.tile([128, 4 * (T2 + 2)], dt.bfloat16,
                                     tag="ek")
                    nc.sync.dma_start(out=ek[:], in_=ecm[b])
                    self.ek = ek
                self = u_ek  # carrier for closures

                h1k = []

                def u_key_m(m):
                    def f():
                        ps = cpool.tile([128, T2], f32, tag="cps")
                        i = 0
                        for d in range(3):
                            for k in range(4):
                                nc.tensor.matmul(
                                    ps[:], kw1(d, k, m),
                                    self.ek[:, k * (T2 + 2) + d:
                                            k * (T2 + 2) + d + T2],
                                    start=(i == 0), stop=(i == 11))
                                i += 1
                        h = h1kpool.tile([128, T2], dt.bfloat16,
                                         tag=f"h1k{m}")
                        relu_copy(h[:], ps[:], kb1sb[:, m:m + 1])
                        h1k.append(h)
                    return f

                def u_key_tail():
                    ps2 = cpool.tile([C_ATT, T2], f32, tag="cps")
                    for k in range(8):
                        nc.tensor.matmul(ps2[:],
                                         kW2sb[:, k * C_ATT:(k + 1) * C_ATT],
                                         h1k[k][:], start=(k == 0),
                                         stop=(k == 7))
                    ksb = stat.tile([C_ATT, T2], dt.bfloat16, tag="ksb")
                    nc.scalar.activation(ksb[:], ps2[:], AF.Identity,
                                         bias=kb2sb[:])
                    ps3 = cpool.tile([C_ATT, T2], f32, tag="cps")
                    nc.tensor.matmul(ps3[:], W3sb[:], ksb[:], start=True,
                                     stop=True)
                    sq = stat.tile([C_ATT, T2], dt.bfloat16, tag="sq")
                    nc.vector.tensor_tensor(sq[:], ksb[:], ksb[:], OP.mult)
                    psr = cpool.tile([1, T2], f32, tag="cps")
                    nc.tensor.matmul(psr[:], negT[:], sq[:], start=True,
                                     stop=False)
                    nc.tensor.matmul(psr[:], qb3sb[:], ksb[:], start=False,
                                     stop=True)
                    ka = kaugpool.tile([97, T2], dt.bfloat16, tag="kaug")
                    nc.gpsimd.memset(ka[64:96, :], 0.0)
                    nc.scalar.activation(ka[0:C_ATT, :], ps3[:], AF.Copy)
                    nc.scalar.activation(ka[96:97, :], psr[:], AF.Copy)
                    kaug[b] = ka

                def u_q_dma():
                    qsb = qpool.tile([C_MEL, T1 + 2], dt.bfloat16, tag="qsb")
                    nc.sync.dma_start(out=qsb[:], in_=qTd[b])
                    self.qsb = qsb
                    h2 = h2pool.tile([97, T1], dt.bfloat16, tag="h2aug")
                    nc.gpsimd.memset(h2[64:96, :], 0.0)
                    nc.gpsimd.memset(h2[96:97, :], 1.0)
                    h2aug[b] = h2

                def u_q_chunk(c):
                    def f():
                        h1q = []
                        for mi in range(2):
                            ps = cpool.tile([C_MEL, T2], f32, tag="cps")
                            for d in range(3):
                                nc.tensor.matmul(
                                    ps[:],
                                    qW1sb[:, d * CQ1 + mi * C_MEL:
                                          d * CQ1 + (mi + 1) * C_MEL],
                                    self.qsb[:, c * T2 + d:c * T2 + d + T2],
                                    start=(d == 0), stop=(d == 2))
                            h = qpool.tile([C_MEL, T2], dt.bfloat16,
                                           tag=f"h1q{mi}")
                            relu_copy(h[:], ps[:], qb1sb[:, mi:mi + 1])
                            h1q.append(h)
                        ps2 = cpool.tile([C_MEL, T2], f32, tag="cps")
                        for mi in range(2):
                            nc.tensor.matmul(
                                ps2[:], qW2sb[:, mi * C_MEL:(mi + 1) * C_MEL],
                                h1q[mi][:], start=(mi == 0), stop=(mi == 1))
                        relu_copy(h2aug[b][0:C_ATT, c * T2:(c + 1) * T2],
                                  ps2[:], qb2sb[:])
                    return f

                yield u_ek
                for m in range(8):
                    yield u_key_m(m)
                yield u_key_tail
                yield u_q_dma
                for c in range(4):
                    yield u_q_chunk(c)

            def softmax_pair(b, t):
                """t1 tiles (2t, 2t+1) of batch b as one [128, 2*T2] map."""
                m0 = 2 * t
                sps = spsum.tile([128, 2, T2], f32, tag="sps")
                for j in range(2):
                    nc.tensor.matmul(
                        sps[:, j],
                        h2aug[b][:, (m0 + j) * 128:(m0 + j + 1) * 128],
                        kaug[b][:], start=True, stop=True)
                pp = iopool.tile([128, 2, T2], dt.bfloat16, tag="pp")
                nc.sync.dma_start(out=pp[:], in_=ppd[b, t])
                et = iopool.tile([128, 2, T2], dt.bfloat16, tag="et")
                sums = stat.tile([128, 2], f32, tag="sume")
                for j in range(2):
                    nc.scalar.activation(et[:, j], sps[:, j], AF.Exp,
                                         accum_out=sums[:, j:j + 1])
                r1 = stat.tile([128, 2], f32, tag="r1")
                nc.vector.reciprocal(r1[:], sums[:])
                wt = iopool.tile([128, 2, T2], dt.bfloat16, tag="wt")
                sums2 = stat.tile([128, 2], f32, tag="sumw")
                for j in range(2):
                    nc.vector.scalar_tensor_tensor(
                        wt[:, j], et[:, j], r1[:, j:j + 1], pp[:, j],
                        OP.mult, OP.mult,
                        accum_out=None if any_masked else sums2[:, j:j + 1])
                o1 = iopool.tile([128, 2, T2], dt.bfloat16, tag="o1")
                nc.scalar.activation(o1[:], wt[:], AF.Ln)
                nc.sync.dma_start(out=o1d[b, t], in_=o1[:])
                if any_masked:
                    pm = iopool.tile([128, 2, T2], dt.bfloat16, tag="pmt")
                    nc.sync.dma_start(out=pm[:], in_=pmd[b, t])
                    wm = iopool.tile([128, 2, T2], dt.bfloat16, tag="wm")
                    for j in range(2):
                        nc.vector.scalar_tensor_tensor(
                            wm[:, j], et[:, j], r1[:, j:j + 1], pm[:, j],
                            OP.mult, OP.mult, accum_out=sums2[:, j:j + 1])
                    wsrc = wm
                else:
                    wsrc = wt
                r2 = stat.tile([128, 2], f32, tag="r2")
                nc.vector.reciprocal(r2[:], sums2[:])
                o2 = iopool.tile([128, 2, T2], dt.bfloat16, tag="o2")
                for j in range(2):
                    nc.vector.tensor_scalar(o2[:, j], wsrc[:, j],
                                            r2[:, j:j + 1], None, OP.mult)
                nc.sync.dma_start(out=o2d[b, t], in_=o2[:])

            # ---- schedule: conv(b) interleaved with softmax(b-1) ----
            for b in range(BL):
                units = list(conv_units(b))          # 14 units
                pairs = list(range(NM // 2)) if b > 0 else []
                # weave: a softmax pair after every ~1.5 conv units,
                # starting after the first two units
                wi = 0
                for ui, u in enumerate(units):
                    u()
                    if pairs and ui >= 1 and ui % 2 == 1 and wi < len(pairs):
                        softmax_pair(b - 1, pairs[wi])
                        wi += 1
                for t in pairs[wi:]:
                    softmax_pair(b - 1, t)
            for t in range(NM // 2):
                softmax_pair(BL - 1, t)

    nc.compile()
    return nc


def _prep(inputs):
    """Host-side shard prep. Returns (in_maps, any_masked)."""
    queries = np.asarray(inputs["queries"], np.float32)
    keys = np.asarray(inputs["keys"])
    mask = np.asarray(inputs["mask"]).astype(bool)
    prior = np.asarray(inputs["attn_prior"], np.float32)
    emb = np.asarray(inputs["emb"], np.float32)
    kW1 = np.asarray(inputs["kW1"], np.float32)
    kb1 = np.asarray(inputs["kb1"], np.float32)
    kW2 = np.asarray(inputs["kW2"], np.float32)
    kb2 = np.asarray(inputs["kb2"], np.float32)
    qW1 = np.asarray(inputs["qW1"], np.float32)
    qb1 = np.asarray(inputs["qb1"], np.float32)
    qW2 = np.asarray(inputs["qW2"], np.float32)
    qb2 = np.asarray(inputs["qb2"], np.float32)
    qW3 = np.asarray(inputs["qW3"], np.float32)
    qb3 = np.asarray(inputs["qb3"], np.float32)

    any_masked = not mask.all()

    kW1s = np.ascontiguousarray(
        kW1.reshape(3, 4, 128, C1).transpose(2, 0, 1, 3).reshape(
            128, 12 * C1)).astype(BF16)
    kW2s = np.ascontiguousarray(
        kW2[0].reshape(8, 128, C_ATT).transpose(1, 0, 2).reshape(
            128, 8 * C_ATT)).astype(BF16)
    W3s = np.ascontiguousarray((2.0 * TEMP) * qW3[0].T).astype(BF16)
    qW1s = np.ascontiguousarray(
        qW1.transpose(1, 0, 2).reshape(C_MEL, 3 * CQ1)).astype(BF16)
    qW2s = np.ascontiguousarray(
        qW2[0].reshape(2, C_MEL, C_MEL).transpose(1, 0, 2).reshape(
            C_MEL, 2 * C_MEL)).astype(BF16)
    qb3s = ((2.0 * TEMP) * qb3).reshape(C_ATT, 1).astype(BF16)
    kb1s = np.ascontiguousarray(
        kb1.reshape(8, 128).T).astype(np.float32)
    kb2s = kb2.reshape(C_ATT, 1).astype(np.float32)
    qb1s = np.ascontiguousarray(
        qb1.reshape(2, C_MEL).T).astype(np.float32)
    qb2s = qb2.reshape(C_MEL, 1).astype(np.float32)

    priorp = prior + 1e-8
    shared = dict(kW1=kW1s, kW2=kW2s, W3s=W3s, qW1=qW1s, qW2=qW2s,
                  qb3s=qb3s, kb1=kb1s, kb2=kb2s, qb1=qb1s, qb2=qb2s)

    in_maps = []
    for i in range(NCORES):
        bs = slice(BL * i, BL * (i + 1))
        e = emb[keys[bs]]                        # [BL, T2, EMB]
        e_cm = np.zeros((BL, EMB, T2 + 2), np.float32)
        e_cm[:, :, 1:T2 + 1] = e.transpose(0, 2, 1)
        ecm = np.ascontiguousarray(
            e_cm.reshape(BL, 4, 128, T2 + 2).transpose(0, 2, 1, 3).reshape(
                BL, 128, 4 * (T2 + 2))).astype(BF16)
        qT = np.zeros((BL, C_MEL, T1 + 2), np.float32)
        qT[:, :, 1:T1 + 1] = queries[bs].transpose(0, 2, 1)
        qTs = qT.astype(BF16)
        pp = np.ascontiguousarray(
            priorp[bs].reshape(BL, NM // 2, 2, 128, T2).transpose(
                0, 1, 3, 2, 4)).astype(BF16)
        m = dict(ecm=ecm, qT=qTs, priorp=pp, **shared)
        if any_masked:
            pmv = priorp[bs] * mask[bs, :, 0][:, None, :]
            m["pm"] = np.ascontiguousarray(
                pmv.reshape(BL, NM // 2, 2, 128, T2).transpose(
                    0, 1, 3, 2, 4)).astype(BF16)
        in_maps.append(m)
    return in_maps, any_masked


def _assemble(results):
    out1 = np.empty((B, 1, T1, T2), np.float32)
    out2 = np.empty((B, 1, T1, T2), np.float32)
    for i, r in enumerate(results):
        for name, dst in (("out1", out1), ("out2", out2)):
            a = np.asarray(r[name]).astype(np.float32)
            a = a.reshape(BL, NM // 2, 128, 2, T2).transpose(0, 1, 3, 2, 4)
            dst[BL * i:BL * (i + 1), 0] = a.reshape(BL, T1, T2)
    return out2, out1


def kernel(**inputs):
    from concourse import bass_utils

    in_maps, any_masked = _prep(inputs)
    key = any_masked
    if key not in _cache:
        _cache[key] = _build(any_masked)
    nc = _cache[key]
    res = bass_utils.run_bass_kernel_spmd(
        nc, in_maps, core_ids=list(range(NCORES)))
    return _assemble(res.results)
